# revision 2
# baseline (speedup 1.0000x reference)
"""GraphTransformerLayer Trainium2 kernel (8 NeuronCores, SPMD) — v2.

Design (f32-accurate datapath; the f16-everywhere v1 failed the 2e-2 gate):
 - Nodes sharded across 8 cores; edges owned by dst core, sorted by dst,
   packed into 128-edge tiles that never cross a 128-node chunk.
 - K|V table: f32 rows (K 512B | V 512B), built per owner, all-gathered.
 - Per tile: indirect-DMA gather of K|V rows; one-hot slot matrix (f16,
   exact) broadcasts the chunk's Q to edges via PE matmul in an f16
   hi+lo pair (exact to ~2^-21); scores on DVE in f32; exp on ACT;
   V-weighting in f32 then split to an f16 hi+lo pair for the PE
   scatter matmul (UT accumulated in PSUM as [slot, hi|lo] columns).
 - Per chunk: UT = hi+lo, normalize by the summed exp pair, transpose via
   DMA, Wo matmul in f32, residual, BN2 partial stats.
 - BN2 stats all-reduced; phase C applies BN2 + FFN (f32 matmuls) + residual.
 - BatchNorm1 folded into the QKV weights on the host; cK/cV constants baked
   into the K|V table so Q needs no extra constant columns.
"""

import numpy as np

# ---------------------------------------------------------------- config
N, E, DIM, H = 50000, 800000, 128, 8
HD = DIM // H
C = 8
EPS = 1e-5
CHUNK = 128
DUMMY_SLOT = 200.0
GATHER_B = 16       # edge tiles per indirect-DMA gather op
SMALL_G = 4         # tiles per batched clamp/exp group


def _ceil_div(a, b):
    return (a + b - 1) // b


# ---------------------------------------------------------------- host prep
def _fold_weights(inp):
    f = np.float32
    h = np.asarray(inp["h"], f)
    mu1 = h.mean(0, dtype=np.float64).astype(f)
    var1 = h.var(0, dtype=np.float64).astype(f)
    rstd1 = (1.0 / np.sqrt(var1 + EPS)).astype(f)
    a1 = rstd1 * np.asarray(inp["bn1_g"], f)
    c1 = np.asarray(inp["bn1_b"], f) - mu1 * a1

    Wq = np.asarray(inp["Wq"], f)
    Wk = np.asarray(inp["Wk"], f)
    Wv = np.asarray(inp["Wv"], f)
    Wo = np.asarray(inp["Wo"], f)
    Wq_eff = a1[:, None] * Wq
    Wk_eff = a1[:, None] * Wk
    Wv_eff = a1[:, None] * Wv
    cQ = c1 @ Wq
    cK = c1 @ Wk
    cV = c1 @ Wv

    # Q side pre-scaled by 1/sqrt(HD)=1/4; cK is baked into the K table rows
    rhs_q = (0.25 * Wq_eff).astype(f)          # [128, 128]
    cq_rep = np.tile((0.25 * cQ)[None, :], (128, 1)).astype(f)

    wkv = np.concatenate([Wk_eff, Wv_eff], axis=1).astype(f)  # [128, 256]
    cvkv = np.concatenate([cK, cV])[None, :].astype(f) * np.ones((128, 1), f)

    W1 = np.asarray(inp["W1"], f)
    b1_eff = np.asarray(inp["b1"], f) + np.asarray(inp["bn2_b"], f) @ W1
    W2 = np.asarray(inp["W2"], f)

    cvec = np.zeros((128, 8), f)
    cvec[:, 0] = np.asarray(inp["bo"], f)
    cvec[:, 1] = b1_eff[:128]
    cvec[:, 2] = b1_eff[128:]
    cvec[:, 3] = np.asarray(inp["b2"], f)
    cvec[:, 4] = np.asarray(inp["bn2_g"], f)
    cvec[:, 5] = EPS

    return dict(
        rhs_q=rhs_q,
        cq_rep=cq_rep,
        wkv=wkv,
        cvkv=cvkv,
        wo=Wo.astype(f),
        w1=W1.astype(f),
        w2a=np.ascontiguousarray(W2[:128, :]).astype(f),
        w2b=np.ascontiguousarray(W2[128:, :]).astype(f),
        cvec=cvec,
    )


def _prep_edges(src, dst, n, c_cores, npc, nchunk):
    """Per-core (srcmeta [128,T] i32, slotmeta [128,T] f32) + tiles-per-chunk."""
    owner = dst // npc
    per_core = []
    counts = np.zeros((c_cores, nchunk), np.int64)
    for c in range(c_cores):
        m = owner == c
        es, ed = src[m], dst[m]
        order = np.argsort(ed, kind="stable")
        es, ed = es[order], ed[order]
        local = ed - c * npc
        cid = local // CHUNK
        counts[c] = np.bincount(cid, minlength=nchunk)
        per_core.append((es, local))
    tpc = [max(1, int(_ceil_div(int(counts[:, mm].max()), 128))) for mm in range(nchunk)]
    T = int(sum(tpc))

    srcmeta = np.zeros((c_cores, 128, T), np.int32)
    slotmeta = np.full((c_cores, 128, T), DUMMY_SLOT, np.float32)
    tile_of_chunk = np.cumsum([0] + tpc)
    for c in range(c_cores):
        es, local = per_core[c]
        cid = local // CHUNK
        slot = (local % CHUNK).astype(np.float32)
        start = np.searchsorted(cid, np.arange(nchunk))
        end = np.searchsorted(cid, np.arange(nchunk), side="right")
        for mm in range(nchunk):
            cnt = end[mm] - start[mm]
            t0 = tile_of_chunk[mm]
            full = es[start[mm] : end[mm]]
            sl = slot[start[mm] : end[mm]]
            ntile = _ceil_div(max(cnt, 1), 128)
            assert ntile <= tpc[mm]
            for j in range(ntile):
                a, b = 128 * j, min(128 * (j + 1), cnt)
                srcmeta[c, : b - a, t0 + j] = full[a:b]
                slotmeta[c, : b - a, t0 + j] = sl[a:b]
    return srcmeta, slotmeta, tpc, T


# ---------------------------------------------------------------- bass build
def _build(cfg, stage=3):
    import concourse.bacc as bacc
    import concourse.mybir as mybir
    import concourse.tile as tile
    from concourse import bass

    n, c_cores, npc = cfg["N"], cfg["C"], cfg["NPC"]
    nchunk, npad = cfg["NCHUNK"], cfg["NCHUNK"] * CHUNK
    tpc, T, B = cfg["tpc"], cfg["T"], cfg["B"]
    f32, f16, i32 = mybir.dt.float32, mybir.dt.float16, mybir.dt.int32
    AF = mybir.ActivationFunctionType
    OP = mybir.AluOpType

    nc = bacc.Bacc("TRN2", target_bir_lowering=False, debug=False, num_devices=c_cores)
    dti = lambda name, shape, dt=f32: nc.dram_tensor(name, shape, dt, kind="ExternalInput").ap()
    hT_d = dti("hT", (128, npad))
    srcm_d = dti("srcmeta", (128, T), i32)
    slotm_d = dti("slotmeta", (128, T))
    slotmT_d = dti("slotmetaT", (1, T * 128), mybir.dt.float16)
    rhs_q_d = dti("rhs_q", (128, 128))
    cq_rep_d = dti("cq_rep", (128, 128))
    wkv_d = dti("wkv", (128, 256))
    cvkv_d = dti("cvkv", (128, 256))
    wo_d = dti("wo", (128, 128))
    w1_d = dti("w1", (128, 256))
    w2a_d = dti("w2a", (128, 128))
    w2b_d = dti("w2b", (128, 128))
    cvec_d = dti("cvec", (128, 8))
    outT_d = nc.dram_tensor("outT", (128, npad), f32, kind="ExternalOutput").ap()

    from contextlib import ExitStack

    with tile.TileContext(nc) as tc, ExitStack() as ctx:
        persist = ctx.enter_context(tc.tile_pool(name="persist", bufs=1))
        ring = ctx.enter_context(tc.tile_pool(name="ring", bufs=3))
        ringK = ctx.enter_context(tc.tile_pool(name="ringK", bufs=10))
        ringG = ctx.enter_context(tc.tile_pool(name="ringG", bufs=2))
        psum = ctx.enter_context(tc.tile_pool(name="psum", bufs=2, space="PSUM"))
        psacc = ctx.enter_context(tc.tile_pool(name="psacc", bufs=2, space="PSUM"))
        dram = ctx.enter_context(tc.tile_pool(name="dram", bufs=1, space="DRAM"))

        # ---------------- persistent loads
        hT = persist.tile([128, npad], f32)
        nc.sync.dma_start(hT[:], hT_d[:, :])
        srcm = persist.tile([128, T], i32)
        nc.sync.dma_start(srcm[:], srcm_d[:, :])
        slotm = persist.tile([128, T], f32)
        nc.sync.dma_start(slotm[:], slotm_d[:, :])
        rhs_q = persist.tile([128, 128], f32)
        nc.sync.dma_start(rhs_q[:], rhs_q_d[:, :])
        cq_rep = persist.tile([128, 128], f32)
        nc.sync.dma_start(cq_rep[:], cq_rep_d[:, :])
        wkv = persist.tile([128, 256], f32)
        nc.sync.dma_start(wkv[:], wkv_d[:, :])
        cvkv = persist.tile([128, 256], f32)
        nc.sync.dma_start(cvkv[:], cvkv_d[:, :])
        wo = persist.tile([128, 128], f32)
        nc.sync.dma_start(wo[:], wo_d[:, :])
        w1 = persist.tile([128, 256], f32)
        nc.sync.dma_start(w1[:], w1_d[:, :])
        w2a = persist.tile([128, 128], f32)
        nc.sync.dma_start(w2a[:], w2a_d[:, :])
        w2b = persist.tile([128, 128], f32)
        nc.sync.dma_start(w2b[:], w2b_d[:, :])
        cvec = persist.tile([128, 8], f32)
        nc.sync.dma_start(cvec[:], cvec_d[:, :])

        iota_i = persist.tile([128, 128], i32)
        nc.gpsimd.iota(iota_i[:], pattern=[[1, 128]], base=0, channel_multiplier=0)
        iota_f = persist.tile([128, 128], f32)
        nc.vector.tensor_copy(iota_f[:], iota_i[:])
        iotaP_i = persist.tile([128, 1], i32)
        nc.gpsimd.iota(iotaP_i[:], pattern=[[1, 1]], base=0, channel_multiplier=1)
        iotaP_f = persist.tile([128, 1], f32)
        nc.vector.tensor_copy(iotaP_f[:], iotaP_i[:])
        ident_f = persist.tile([128, 128], f32)
        nc.vector.tensor_tensor(
            out=ident_f[:],
            in0=iota_i[:],
            in1=iotaP_i[:].to_broadcast([128, 128]),
            op=OP.is_equal,
        )

        h2T = persist.tile([128, npad], f32)
        s1p = persist.tile([128, nchunk], f32)
        s2p = persist.tile([128, nchunk], f32)

        kv_own = dram.tile([npc, 256], f32)
        kv_full = nc.dram_tensor("kv_full_sh", (n, 256), f32, kind="Internal", addr_space="Shared").ap()

        # ---------------- phase A: K|V rows (+cK|cV), all-gather
        for m in range(nchunk):
            cn = min(CHUNK, npc - m * CHUNK)
            kvp = psum.tile([128, 256], f32, tag="pa")
            nc.tensor.matmul(out=kvp[:], lhsT=hT[:, m * 128 : (m + 1) * 128], rhs=wkv[:], start=True, stop=True)
            kvs = ring.tile([128, 256], f32, tag="kvs")
            nc.vector.tensor_tensor(out=kvs[:], in0=kvp[:], in1=cvkv[:], op=OP.add)
            nc.sync.dma_start(kv_own[m * 128 : m * 128 + cn, :], kvs[:cn, :])
        if c_cores > 1:
            nc.gpsimd.collective_compute(
                "AllGather",
                mybir.AluOpType.bypass,
                replica_groups=[list(range(c_cores))],
                ins=[kv_own[:].opt()],
                outs=[kv_full[:].opt()],
            )
            kv_src = kv_full
        else:
            kv_src = kv_own

        # ---------------- phase B: edge attention
        t = 0
        kvg = None
        for m in range(nchunk):
            cn = min(CHUNK, npc - m * CHUNK)
            ntile = tpc[m]
            # Q chunk (pre-scaled 1/4), f16 hi/lo pair
            qp = psum.tile([128, 128], f32, tag="pa")
            nc.tensor.matmul(out=qp[:], lhsT=hT[:, m * 128 : (m + 1) * 128], rhs=rhs_q[:], start=True, stop=True)
            q32 = ring.tile([128, 128], f32, tag="q32")
            nc.vector.tensor_tensor(out=q32[:], in0=qp[:], in1=cq_rep[:], op=OP.add)
            UT = psacc.tile([128, 136], f32, tag="acc")
            if stage >= 3:
                t0c = t
                slotrep = ringG.tile([128, max(tpc) * 128], f16, tag="slotrep")
                nc.sync.dma_start(
                    slotrep[:, : ntile * 128],
                    slotmT_d[0:1, t * 128 : (t + ntile) * 128].to_broadcast([128, ntile * 128]),
                )
                for j0 in range(0, ntile, SMALL_G):
                    ng = min(SMALL_G, ntile - j0)
                    prod_b = ringG.tile([128, SMALL_G * 128], f32, tag="prod")
                    scr_b = ringG.tile([128, SMALL_G * 8], f32, tag="scr")
                    kvg_l, oh32_l = [], []
                    for g in range(ng):
                        tt = t + g
                        kvg = ringK.tile([128, 256], f32, tag="kvg")
                        nc.gpsimd.indirect_dma_start(
                            out=kvg[:],
                            out_offset=None,
                            in_=kv_src[:],
                            in_offset=bass.IndirectOffsetOnAxis(ap=srcm[:, tt : tt + 1], axis=0),
                        )
                        kvg_l.append(kvg)

                        oh32 = ringK.tile([128, 128], f32, tag="oh32")
                        nc.vector.tensor_tensor(
                            out=oh32[:],
                            in0=slotm[:, tt : tt + 1].to_broadcast([128, 128]),
                            in1=iota_f[:],
                            op=OP.is_equal,
                        )
                        oh32_l.append(oh32)
                        ohT = ring.tile([128, 128], f32, tag="ohTs")
                        nc.vector.tensor_tensor(
                            out=ohT[:],
                            in0=slotrep[:, (tt - t0c) * 128 : (tt - t0c + 1) * 128],
                            in1=iotaP_f[:].to_broadcast([128, 128]),
                            op=OP.is_equal,
                        )

                        qd = psum.tile([128, 128], f32, tag="qd")
                        nc.tensor.matmul(out=qd[:], lhsT=ohT[:], rhs=q32[:], start=True, stop=True)

                        nc.vector.tensor_tensor(
                            out=prod_b[:, g * 128 : (g + 1) * 128],
                            in0=kvg[:, 0:128], in1=qd[:], op=OP.mult,
                        )
                    nc.vector.tensor_reduce(
                        out=scr_b[:, : ng * 8],
                        in_=prod_b[:, : ng * 128].rearrange("p (gh d) -> p gh d", d=16),
                        op=OP.add,
                        axis=mybir.AxisListType.X,
                    )
                    nc.vector.tensor_scalar(
                        out=scr_b[:, : ng * 8], in0=scr_b[:, : ng * 8],
                        scalar1=5.0, scalar2=-5.0, op0=OP.min, op1=OP.max,
                    )
                    for g in range(ng):
                        j = j0 + g
                        ms32 = ring.tile([128, 136], f32, tag="ms32")
                        nc.scalar.activation(
                            out=ms32[:, 128:136], in_=scr_b[:, g * 8 : (g + 1) * 8], func=AF.Exp
                        )
                        nc.vector.tensor_tensor(
                            out=ms32[:, 0:128].rearrange("p (h d) -> p h d", h=8),
                            in0=kvg_l[g][:, 128:256].rearrange("p (h d) -> p h d", h=8),
                            in1=ms32[:, 128:136].unsqueeze(-1).to_broadcast([128, 8, 16]),
                            op=OP.mult,
                        )
                        nc.tensor.matmul(
                            out=UT[:], lhsT=oh32_l[g][:], rhs=ms32[:],
                            start=(j == 0), stop=(j == ntile - 1),
                        )
                    t += ng
            else:
                t += ntile

            # ---- chunk finalize
            if stage >= 3:
                deng = ring.tile([128, 8], f32, tag="deng")
                nc.vector.tensor_scalar_max(deng[:], UT[:, 128:136], 1e-30)
                denr = ring.tile([128, 8], f32, tag="denr")
                nc.vector.reciprocal(denr[:], deng[:])
                wv = ring.tile([128, 128], f32, tag="wv")
                nc.vector.tensor_tensor(
                    out=wv[:].rearrange("p (h d) -> p h d", h=8),
                    in0=UT[:, 0:128].rearrange("p (h d) -> p h d", h=8),
                    in1=denr[:].unsqueeze(-1).to_broadcast([128, 8, 16]),
                    op=OP.mult,
                )
                wvTp = psum.tile([128, 128], f32, tag="qd")
                nc.tensor.transpose(wvTp[:], wv[:], ident_f[:])
                wvT = ring.tile([128, 128], f32, tag="wvT")
                nc.scalar.copy(out=wvT[:], in_=wvTp[:])
                h2p = psum.tile([128, 128], f32, tag="pa")
                nc.tensor.matmul(out=h2p[:], lhsT=wo[:], rhs=wvT[:], start=True, stop=True)
                nc.vector.scalar_tensor_tensor(
                    out=h2T[:, m * 128 : (m + 1) * 128],
                    in0=h2p[:],
                    scalar=cvec[:, 0:1],
                    op0=OP.add,
                    in1=hT[:, m * 128 : (m + 1) * 128],
                    op1=OP.add,
                )
            else:
                nc.vector.tensor_copy(h2T[:, m * 128 : (m + 1) * 128], hT[:, m * 128 : (m + 1) * 128])
            nc.vector.tensor_reduce(
                out=s1p[:, m : m + 1], in_=h2T[:, m * 128 : m * 128 + cn], op=OP.add,
                axis=mybir.AxisListType.X,
            )
            junk = ring.tile([128, 128], f32, tag="junk")
            nc.scalar.activation(
                out=junk[:, :cn],
                in_=h2T[:, m * 128 : m * 128 + cn],
                func=AF.Square,
                accum_out=s2p[:, m : m + 1],
            )

        if stage < 1:
            # stage 0: bail out after phase A/B skeleton — just write h2T
            kvchk = ring.tile([128, 256], f32, tag="kvchk")
            nc.sync.dma_start(kvchk[:], kv_src[0:128, :])
            for m in range(nchunk):
                cn = min(CHUNK, npc - m * CHUNK)
                nc.sync.dma_start(outT_d[:, m * 128 : m * 128 + cn], h2T[:, m * 128 : m * 128 + cn])
            nc.sync.dma_start(outT_d[:, 0:128], kvchk[:, 0:128])
        _skip = stage < 1

        # ---------------- BN2 stats all-reduce
        stats = ring.tile([128, 2], f32, tag="stats")
        nc.vector.tensor_reduce(out=stats[:, 0:1], in_=s1p[:], op=OP.add, axis=mybir.AxisListType.X)
        nc.vector.tensor_reduce(out=stats[:, 1:2], in_=s2p[:], op=OP.add, axis=mybir.AxisListType.X)
        if c_cores > 1:
            st_in = dram.tile([128, 2], f32)
            st_out = nc.dram_tensor("st_out_sh", (128, 2), f32, kind="Internal", addr_space="Shared").ap()
            nc.sync.dma_start(st_in[:], stats[:])
            nc.gpsimd.collective_compute(
                "AllReduce",
                mybir.AluOpType.add,
                replica_groups=[list(range(c_cores))],
                ins=[st_in[:].opt()],
                outs=[st_out[:].opt()],
            )
            stg = ring.tile([128, 2], f32, tag="stg")
            nc.sync.dma_start(stg[:], st_out[:])
        else:
            stg = stats
        mean = ring.tile([128, 1], f32, tag="mean")
        nc.vector.tensor_scalar_mul(mean[:], stg[:, 0:1], 1.0 / n)
        ex2 = ring.tile([128, 1], f32, tag="ex2")
        nc.vector.tensor_scalar_mul(ex2[:], stg[:, 1:2], 1.0 / n)
        var = ring.tile([128, 1], f32, tag="var")
        nc.vector.tensor_tensor(out=var[:], in0=mean[:], in1=mean[:], op=OP.mult)
        nc.vector.tensor_tensor(out=var[:], in0=ex2[:], in1=var[:], op=OP.subtract)
        std = ring.tile([128, 1], f32, tag="std")
        nc.scalar.activation(out=std[:], in_=var[:], func=AF.Sqrt, bias=cvec[:, 5:6])
        rstd = ring.tile([128, 1], f32, tag="rstd")
        nc.vector.reciprocal(rstd[:], std[:])
        sc2 = ring.tile([128, 1], f32, tag="sc2")
        nc.vector.tensor_tensor(out=sc2[:], in0=rstd[:], in1=cvec[:, 4:5], op=OP.mult)

        # ---------------- phase C: BN2 apply + FFN + residual
        for m in range(nchunk):
            cn = min(CHUNK, npc - m * CHUNK)
            u = ring.tile([128, 128], f32, tag="u")
            nc.vector.scalar_tensor_tensor(
                out=u[:],
                in0=h2T[:, m * 128 : (m + 1) * 128],
                scalar=mean[:],
                op0=OP.subtract,
                in1=sc2[:].to_broadcast([128, 128]),
                op1=OP.mult,
            )
            y1a = psum.tile([128, 128], f32, tag="pa")
            nc.tensor.matmul(out=y1a[:], lhsT=w1[:, :128], rhs=u[:], start=True, stop=True)
            y1b = psum.tile([128, 128], f32, tag="qd")
            nc.tensor.matmul(out=y1b[:], lhsT=w1[:, 128:256], rhs=u[:], start=True, stop=True)
            r1a = ring.tile([128, 128], f32, tag="r1a")
            nc.scalar.activation(out=r1a[:], in_=y1a[:], func=AF.Relu, bias=cvec[:, 1:2])
            r1b = ring.tile([128, 128], f32, tag="r1b")
            nc.scalar.activation(out=r1b[:], in_=y1b[:], func=AF.Relu, bias=cvec[:, 2:3])
            h3 = psum.tile([128, 128], f32, tag="qd")
            nc.tensor.matmul(out=h3[:], lhsT=w2a[:], rhs=r1a[:], start=True, stop=False)
            nc.tensor.matmul(out=h3[:], lhsT=w2b[:], rhs=r1b[:], start=False, stop=True)
            outc = ring.tile([128, 128], f32, tag="outc")
            nc.vector.scalar_tensor_tensor(
                out=outc[:],
                in0=h3[:],
                scalar=cvec[:, 3:4],
                op0=OP.add,
                in1=h2T[:, m * 128 : (m + 1) * 128],
                op1=OP.add,
            )
            nc.sync.dma_start(outT_d[:, m * 128 : m * 128 + cn], outc[:, :cn])

    nc.compile()
    return nc


# ---------------------------------------------------------------- entry
def _make_cfg(n, e, c_cores, src, dst, b=GATHER_B):
    npc = n // c_cores
    nchunk = _ceil_div(npc, CHUNK)
    srcmeta, slotmeta, tpc, T = _prep_edges(src, dst, n, c_cores, npc, nchunk)
    cfg = dict(N=n, E=e, C=c_cores, NPC=npc, NCHUNK=nchunk, tpc=tpc, T=T, B=b)
    return cfg, srcmeta, slotmeta


def _make_in_maps(cfg, srcmeta, slotmeta, inp):
    f = np.float32
    n, c_cores, npc = cfg["N"], cfg["C"], cfg["NPC"]
    npad = cfg["NCHUNK"] * CHUNK
    w = _fold_weights(inp)
    h = np.asarray(inp["h"], f)
    in_maps = []
    for c in range(c_cores):
        hT = np.zeros((128, npad), f)
        hT[:, :npc] = h[c * npc : (c + 1) * npc, :].T
        m = dict(
            hT=hT, srcmeta=srcmeta[c], slotmeta=slotmeta[c],
            slotmetaT=np.ascontiguousarray(slotmeta[c].T).astype(np.float16).reshape(1, -1),
            **w,
        )
        in_maps.append(m)
    return in_maps


_CACHE = {}
_PROFILE = False
_LAST_RES = None


def kernel(**inputs):
    global _LAST_RES
    from concourse.bass_utils import run_bass_kernel_spmd

    src = np.asarray(inputs["src"]).astype(np.int32)
    dst = np.asarray(inputs["dst"]).astype(np.int32)
    cfg, srcmeta, slotmeta = _make_cfg(N, E, C, src, dst)
    key = ("full", tuple(cfg["tpc"]))
    if key not in _CACHE:
        _CACHE[key] = _build(cfg)
    nc = _CACHE[key]
    in_maps = _make_in_maps(cfg, srcmeta, slotmeta, inputs)
    res = run_bass_kernel_spmd(nc, in_maps, core_ids=list(range(C)), trace=_PROFILE)
    _LAST_RES = res
    npc = cfg["NPC"]
    out = np.empty((N, DIM), np.float32)
    for c in range(C):
        out[c * npc : (c + 1) * npc, :] = res.results[c]["outT"][:, :npc].T
    return out


# revision 3
# speedup vs baseline: 1.0595x; 1.0595x over previous
"""GraphTransformerLayer Trainium2 kernel (8 NeuronCores, SPMD) — v2.

Design (f32-accurate datapath; the f16-everywhere v1 failed the 2e-2 gate):
 - Nodes sharded across 8 cores; edges owned by dst core, sorted by dst,
   packed into 128-edge tiles that never cross a 128-node chunk.
 - K|V table: f32 rows (K 512B | V 512B), built per owner, all-gathered.
 - Per tile: indirect-DMA gather of K|V rows; one-hot slot matrix (f16,
   exact) broadcasts the chunk's Q to edges via PE matmul in an f16
   hi+lo pair (exact to ~2^-21); scores on DVE in f32; exp on ACT;
   V-weighting in f32 then split to an f16 hi+lo pair for the PE
   scatter matmul (UT accumulated in PSUM as [slot, hi|lo] columns).
 - Per chunk: UT = hi+lo, normalize by the summed exp pair, transpose via
   DMA, Wo matmul in f32, residual, BN2 partial stats.
 - BN2 stats all-reduced; phase C applies BN2 + FFN (f32 matmuls) + residual.
 - BatchNorm1 folded into the QKV weights on the host; cK/cV constants baked
   into the K|V table so Q needs no extra constant columns.
"""

import numpy as np

# ---------------------------------------------------------------- config
N, E, DIM, H = 50000, 800000, 128, 8
HD = DIM // H
C = 8
EPS = 1e-5
CHUNK = 128
DUMMY_SLOT = 200.0
GATHER_B = 16       # edge tiles per indirect-DMA gather op
SMALL_G = 4         # tiles per batched clamp/exp group
NBAND = 4           # all-gather split (kv_full is band-major)


def _ceil_div(a, b):
    return (a + b - 1) // b


# ---------------------------------------------------------------- host prep
def _fold_weights(inp):
    f = np.float32
    h = np.asarray(inp["h"], f)
    mu1 = h.mean(0, dtype=np.float64).astype(f)
    var1 = h.var(0, dtype=np.float64).astype(f)
    rstd1 = (1.0 / np.sqrt(var1 + EPS)).astype(f)
    a1 = rstd1 * np.asarray(inp["bn1_g"], f)
    c1 = np.asarray(inp["bn1_b"], f) - mu1 * a1

    Wq = np.asarray(inp["Wq"], f)
    Wk = np.asarray(inp["Wk"], f)
    Wv = np.asarray(inp["Wv"], f)
    Wo = np.asarray(inp["Wo"], f)
    Wq_eff = a1[:, None] * Wq
    Wk_eff = a1[:, None] * Wk
    Wv_eff = a1[:, None] * Wv
    cQ = c1 @ Wq
    cK = c1 @ Wk
    cV = c1 @ Wv

    # Q side pre-scaled by 1/sqrt(HD)=1/4; cK is baked into the K table rows
    rhs_q = (0.25 * Wq_eff).astype(f)          # [128, 128]
    cq_rep = np.tile((0.25 * cQ)[None, :], (128, 1)).astype(f)

    wkv = np.concatenate([Wk_eff, Wv_eff], axis=1).astype(f)  # [128, 256]
    cvkv = np.concatenate([cK, cV])[None, :].astype(f) * np.ones((128, 1), f)

    W1 = np.asarray(inp["W1"], f)
    b1_eff = np.asarray(inp["b1"], f) + np.asarray(inp["bn2_b"], f) @ W1
    W2 = np.asarray(inp["W2"], f)

    cvec = np.zeros((128, 8), f)
    cvec[:, 0] = np.asarray(inp["bo"], f)
    cvec[:, 1] = b1_eff[:128]
    cvec[:, 2] = b1_eff[128:]
    cvec[:, 3] = np.asarray(inp["b2"], f)
    cvec[:, 4] = np.asarray(inp["bn2_g"], f)
    cvec[:, 5] = EPS

    return dict(
        rhs_q=rhs_q,
        cq_rep=cq_rep,
        wkv=wkv,
        cvkv=cvkv,
        wo=Wo.astype(f),
        w1=W1.astype(f),
        w2a=np.ascontiguousarray(W2[:128, :]).astype(f),
        w2b=np.ascontiguousarray(W2[128:, :]).astype(f),
        cvec=cvec,
    )


def _band_bounds(npc):
    bs = _ceil_div(npc, NBAND)
    return [min(b * bs, npc) for b in range(NBAND + 1)]


def _bandmap(v, npc, c_cores):
    """Global node id -> row in the band-major all-gathered table."""
    B = np.array(_band_bounds(npc))
    c, r = v // npc, v % npc
    band = np.searchsorted(B, r, side="right") - 1
    base = B[band] * c_cores + c * (B[band + 1] - B[band]) + (r - B[band])
    return base.astype(np.int32)


def _prep_edges(src, dst, n, c_cores, npc, nchunk):
    """Per-core (srcmeta [128,T] i32, slotmeta [128,T] f32) + tiles-per-chunk."""
    owner = dst // npc
    per_core = []
    counts = np.zeros((c_cores, nchunk), np.int64)
    loccnt = np.zeros((c_cores, nchunk), np.int64)
    for c in range(c_cores):
        m = owner == c
        es, ed = src[m], dst[m]
        local = ed - c * npc
        cid = local // CHUNK
        # sort by (chunk, remote-src-flag) so local-src edges lead each chunk
        is_rem = (es // npc) != c
        order = np.lexsort((is_rem, cid))
        es, local, cid2 = es[order], local[order], cid[order]
        counts[c] = np.bincount(cid2, minlength=nchunk)
        for mm in range(nchunk):
            sel = cid2 == mm
            loccnt[c, mm] = int(((es[sel] // npc) == c).sum())
        per_core.append((es, local))
    tpc = [max(1, int(_ceil_div(int(counts[:, mm].max()), 128))) for mm in range(nchunk)]
    # full local tiles guaranteed on every core (shared SPMD program)
    nloc = [min(int(loccnt[:, mm].min()) // 128, tpc[mm] - 1) for mm in range(nchunk)]
    T = int(sum(tpc))

    srcmeta = np.zeros((c_cores, 128, T), np.int32)
    slotmeta = np.full((c_cores, 128, T), DUMMY_SLOT, np.float32)
    tile_of_chunk = np.cumsum([0] + tpc)
    for c in range(c_cores):
        es, local = per_core[c]
        cid = local // CHUNK
        slot = (local % CHUNK).astype(np.float32)
        start = np.searchsorted(cid, np.arange(nchunk))
        end = np.searchsorted(cid, np.arange(nchunk), side="right")
        for mm in range(nchunk):
            cnt = end[mm] - start[mm]
            t0 = tile_of_chunk[mm]
            full = es[start[mm] : end[mm]]
            sl = slot[start[mm] : end[mm]]
            ntile = _ceil_div(max(cnt, 1), 128)
            assert ntile <= tpc[mm]
            for j in range(ntile):
                a, b = 128 * j, min(128 * (j + 1), cnt)
                idxs = full[a:b]
                if j < nloc[mm]:
                    idxs = idxs - c * npc  # local table indices
                srcmeta[c, : b - a, t0 + j] = idxs
                slotmeta[c, : b - a, t0 + j] = sl[a:b]
    return srcmeta, slotmeta, tpc, T, nloc


# ---------------------------------------------------------------- bass build
def _build(cfg, stage=3):
    import concourse.bacc as bacc
    import concourse.mybir as mybir
    import concourse.tile as tile
    from concourse import bass

    n, c_cores, npc = cfg["N"], cfg["C"], cfg["NPC"]
    nchunk, npad = cfg["NCHUNK"], cfg["NCHUNK"] * CHUNK
    tpc, T, B = cfg["tpc"], cfg["T"], cfg["B"]
    nloc = cfg["nloc"]
    f32, f16, i32 = mybir.dt.float32, mybir.dt.float16, mybir.dt.int32
    AF = mybir.ActivationFunctionType
    OP = mybir.AluOpType

    nc = bacc.Bacc("TRN2", target_bir_lowering=False, debug=False, num_devices=c_cores)
    dti = lambda name, shape, dt=f32: nc.dram_tensor(name, shape, dt, kind="ExternalInput").ap()
    hT_d = dti("hT", (128, npad))
    srcm_d = dti("srcmeta", (128, T), i32)
    slotm_d = dti("slotmeta", (128, T))
    slotmT_d = dti("slotmetaT", (1, T * 128), mybir.dt.float16)
    rhs_q_d = dti("rhs_q", (128, 128))
    cq_rep_d = dti("cq_rep", (128, 128))
    wkv_d = dti("wkv", (128, 256))
    cvkv_d = dti("cvkv", (128, 256))
    wo_d = dti("wo", (128, 128))
    w1_d = dti("w1", (128, 256))
    w2a_d = dti("w2a", (128, 128))
    w2b_d = dti("w2b", (128, 128))
    cvec_d = dti("cvec", (128, 8))
    outT_d = nc.dram_tensor("outT", (128, npad), f32, kind="ExternalOutput").ap()

    from contextlib import ExitStack

    with tile.TileContext(nc) as tc, ExitStack() as ctx:
        persist = ctx.enter_context(tc.tile_pool(name="persist", bufs=1))
        ring = ctx.enter_context(tc.tile_pool(name="ring", bufs=3))
        ringK = ctx.enter_context(tc.tile_pool(name="ringK", bufs=12))
        ringG = ctx.enter_context(tc.tile_pool(name="ringG", bufs=3))
        psum = ctx.enter_context(tc.tile_pool(name="psum", bufs=2, space="PSUM"))
        psacc = ctx.enter_context(tc.tile_pool(name="psacc", bufs=2, space="PSUM"))
        dram = ctx.enter_context(tc.tile_pool(name="dram", bufs=1, space="DRAM"))

        # ---------------- persistent loads
        hT = persist.tile([128, npad], f32)
        nc.sync.dma_start(hT[:], hT_d[:, :])
        srcm = persist.tile([128, T], i32)
        nc.sync.dma_start(srcm[:], srcm_d[:, :])
        slotm = persist.tile([128, T], f32)
        nc.sync.dma_start(slotm[:], slotm_d[:, :])
        rhs_q = persist.tile([128, 128], f32)
        nc.sync.dma_start(rhs_q[:], rhs_q_d[:, :])
        cq_rep = persist.tile([128, 128], f32)
        nc.sync.dma_start(cq_rep[:], cq_rep_d[:, :])
        wkv = persist.tile([128, 256], f32)
        nc.sync.dma_start(wkv[:], wkv_d[:, :])
        cvkv = persist.tile([128, 256], f32)
        nc.sync.dma_start(cvkv[:], cvkv_d[:, :])
        wo = persist.tile([128, 128], f32)
        nc.sync.dma_start(wo[:], wo_d[:, :])
        w1 = persist.tile([128, 256], f32)
        nc.sync.dma_start(w1[:], w1_d[:, :])
        w2a = persist.tile([128, 128], f32)
        nc.sync.dma_start(w2a[:], w2a_d[:, :])
        w2b = persist.tile([128, 128], f32)
        nc.sync.dma_start(w2b[:], w2b_d[:, :])
        cvec = persist.tile([128, 8], f32)
        nc.sync.dma_start(cvec[:], cvec_d[:, :])

        iota_i = persist.tile([128, 128], i32)
        nc.gpsimd.iota(iota_i[:], pattern=[[1, 128]], base=0, channel_multiplier=0)
        iota_f = persist.tile([128, 128], f32)
        nc.vector.tensor_copy(iota_f[:], iota_i[:])
        iotaP_i = persist.tile([128, 1], i32)
        nc.gpsimd.iota(iotaP_i[:], pattern=[[1, 1]], base=0, channel_multiplier=1)
        iotaP_f = persist.tile([128, 1], f32)
        nc.vector.tensor_copy(iotaP_f[:], iotaP_i[:])
        ident_f = persist.tile([128, 128], f32)
        nc.vector.tensor_tensor(
            out=ident_f[:],
            in0=iota_i[:],
            in1=iotaP_i[:].to_broadcast([128, 128]),
            op=OP.is_equal,
        )

        h2T = persist.tile([128, npad], f32)
        s1p = persist.tile([128, nchunk], f32)
        s2p = persist.tile([128, nchunk], f32)
        utloc = persist.tile([128, nchunk * 136], f32)
        nc.vector.memset(utloc[:], 0.0)

        kv_own = dram.tile([npc, 256], f32)
        kv_full = nc.dram_tensor("kv_full_sh", (n, 256), f32, kind="Internal", addr_space="Shared").ap()

        # ---------------- phase A: K|V rows (+cK|cV), all-gather
        for m in range(nchunk):
            cn = min(CHUNK, npc - m * CHUNK)
            kvp = psum.tile([128, 256], f32, tag="pa")
            nc.tensor.matmul(out=kvp[:], lhsT=hT[:, m * 128 : (m + 1) * 128], rhs=wkv[:], start=True, stop=True)
            kvs = ring.tile([128, 256], f32, tag="kvs")
            nc.vector.tensor_tensor(out=kvs[:], in0=kvp[:], in1=cvkv[:], op=OP.add)
            nc.sync.dma_start(kv_own[m * 128 : m * 128 + cn, :], kvs[:cn, :])
        tile_of_chunk = [0]
        for mm in range(nchunk):
            tile_of_chunk.append(tile_of_chunk[-1] + tpc[mm])

        def emit_q32(m):
            qp = psum.tile([128, 128], f32, tag="pa")
            nc.tensor.matmul(out=qp[:], lhsT=hT[:, m * 128 : (m + 1) * 128], rhs=rhs_q[:], start=True, stop=True)
            q32 = ring.tile([128, 128], f32, tag="q32")
            nc.vector.tensor_tensor(out=q32[:], in0=qp[:], in1=cq_rep[:], op=OP.add)
            return q32

        def emit_tiles(m, jstart, jend, table, q32, UT, psum_start, psum_stop):
            """Per-tile attention pipeline for tiles [jstart, jend) of chunk m."""
            t0c = tile_of_chunk[m]
            ntile = jend - jstart
            slotrep = ringG.tile([128, max(tpc) * 128], f16, tag="slotrep")
            nc.sync.dma_start(
                slotrep[:, : ntile * 128],
                slotmT_d[0:1, (t0c + jstart) * 128 : (t0c + jend) * 128].to_broadcast([128, ntile * 128]),
            )
            for j0 in range(jstart, jend, SMALL_G):
                ng = min(SMALL_G, jend - j0)
                prod_b = ringG.tile([128, SMALL_G * 128], f32, tag="prod")
                scr_b = ringG.tile([128, SMALL_G * 8], f32, tag="scr")
                kvg_l, oh32_l = [], []
                for g in range(ng):
                    tt = t0c + j0 + g
                    kvg = ringK.tile([128, 256], f32, tag="kvg")
                    nc.gpsimd.indirect_dma_start(
                        out=kvg[:],
                        out_offset=None,
                        in_=table[:],
                        in_offset=bass.IndirectOffsetOnAxis(ap=srcm[:, tt : tt + 1], axis=0),
                    )
                    kvg_l.append(kvg)

                    oh32 = ringK.tile([128, 128], f32, tag="oh32")
                    nc.vector.tensor_tensor(
                        out=oh32[:],
                        in0=slotm[:, tt : tt + 1].to_broadcast([128, 128]),
                        in1=iota_f[:],
                        op=OP.is_equal,
                    )
                    oh32_l.append(oh32)
                    ohT = ring.tile([128, 128], f32, tag="ohTs")
                    nc.vector.tensor_tensor(
                        out=ohT[:],
                        in0=slotrep[:, (j0 + g - jstart) * 128 : (j0 + g - jstart + 1) * 128],
                        in1=iotaP_f[:].to_broadcast([128, 128]),
                        op=OP.is_equal,
                    )

                    qd = psum.tile([128, 128], f32, tag="qd")
                    nc.tensor.matmul(out=qd[:], lhsT=ohT[:], rhs=q32[:], start=True, stop=True)

                    nc.vector.tensor_tensor(
                        out=prod_b[:, g * 128 : (g + 1) * 128],
                        in0=kvg[:, 0:128], in1=qd[:], op=OP.mult,
                    )
                nc.vector.tensor_reduce(
                    out=scr_b[:, : ng * 8],
                    in_=prod_b[:, : ng * 128].rearrange("p (gh d) -> p gh d", d=16),
                    op=OP.add,
                    axis=mybir.AxisListType.X,
                )
                nc.vector.tensor_scalar(
                    out=scr_b[:, : ng * 8], in0=scr_b[:, : ng * 8],
                    scalar1=5.0, scalar2=-5.0, op0=OP.min, op1=OP.max,
                )
                for g in range(ng):
                    j = j0 + g
                    ms32 = ring.tile([128, 136], f32, tag="ms32")
                    nc.scalar.activation(
                        out=ms32[:, 128:136], in_=scr_b[:, g * 8 : (g + 1) * 8], func=AF.Exp
                    )
                    nc.vector.tensor_tensor(
                        out=ms32[:, 0:128].rearrange("p (h d) -> p h d", h=8),
                        in0=kvg_l[g][:, 128:256].rearrange("p (h d) -> p h d", h=8),
                        in1=ms32[:, 128:136].unsqueeze(-1).to_broadcast([128, 8, 16]),
                        op=OP.mult,
                    )
                    nc.tensor.matmul(
                        out=UT[:], lhsT=oh32_l[g][:], rhs=ms32[:],
                        start=(psum_start and j == jstart), stop=(psum_stop and j == jend - 1),
                    )

        # ---------------- pre-pass: all-local tiles, gathered from kv_own
        # (overlaps the all-gather; results parked in utloc)
        if stage >= 3:
            for m in range(nchunk):
                if nloc[m] == 0:
                    continue
                q32p = emit_q32(m)
                UTp = psacc.tile([128, 136], f32, tag="acc")
                emit_tiles(m, 0, nloc[m], kv_own, q32p, UTp, True, True)
                nc.scalar.copy(out=utloc[:, m * 136 : (m + 1) * 136], in_=UTp[:])

        if c_cores > 1:
            nc.gpsimd.collective_compute(
                "AllGather",
                mybir.AluOpType.bypass,
                replica_groups=[list(range(c_cores))],
                ins=[kv_own[:].opt()],
                outs=[kv_full[:].opt()],
            )
            kv_src = kv_full
        else:
            kv_src = kv_own

        # ---------------- phase B: edge attention (remote/mixed tiles)
        for m in range(nchunk):
            cn = min(CHUNK, npc - m * CHUNK)
            ntile = tpc[m]
            q32 = emit_q32(m)
            UT = psacc.tile([128, 136], f32, tag="acc")
            if stage >= 3:
                emit_tiles(m, nloc[m], ntile, kv_src, q32, UT, True, True)

            # ---- chunk finalize
            if stage >= 3:
                utsum = ring.tile([128, 136], f32, tag="utsum")
                nc.vector.tensor_tensor(
                    out=utsum[:], in0=UT[:], in1=utloc[:, m * 136 : (m + 1) * 136], op=OP.add
                )
                deng = ring.tile([128, 8], f32, tag="deng")
                nc.vector.tensor_scalar_max(deng[:], utsum[:, 128:136], 1e-30)
                denr = ring.tile([128, 8], f32, tag="denr")
                nc.vector.reciprocal(denr[:], deng[:])
                wv = ring.tile([128, 128], f32, tag="wv")
                nc.vector.tensor_tensor(
                    out=wv[:].rearrange("p (h d) -> p h d", h=8),
                    in0=utsum[:, 0:128].rearrange("p (h d) -> p h d", h=8),
                    in1=denr[:].unsqueeze(-1).to_broadcast([128, 8, 16]),
                    op=OP.mult,
                )
                wvTp = psum.tile([128, 128], f32, tag="qd")
                nc.tensor.transpose(wvTp[:], wv[:], ident_f[:])
                wvT = ring.tile([128, 128], f32, tag="wvT")
                nc.scalar.copy(out=wvT[:], in_=wvTp[:])
                h2p = psum.tile([128, 128], f32, tag="pa")
                nc.tensor.matmul(out=h2p[:], lhsT=wo[:], rhs=wvT[:], start=True, stop=True)
                nc.vector.scalar_tensor_tensor(
                    out=h2T[:, m * 128 : (m + 1) * 128],
                    in0=h2p[:],
                    scalar=cvec[:, 0:1],
                    op0=OP.add,
                    in1=hT[:, m * 128 : (m + 1) * 128],
                    op1=OP.add,
                )
            else:
                nc.vector.tensor_copy(h2T[:, m * 128 : (m + 1) * 128], hT[:, m * 128 : (m + 1) * 128])
            nc.vector.tensor_reduce(
                out=s1p[:, m : m + 1], in_=h2T[:, m * 128 : m * 128 + cn], op=OP.add,
                axis=mybir.AxisListType.X,
            )
            junk = ring.tile([128, 128], f32, tag="junk")
            nc.scalar.activation(
                out=junk[:, :cn],
                in_=h2T[:, m * 128 : m * 128 + cn],
                func=AF.Square,
                accum_out=s2p[:, m : m + 1],
            )

        if stage < 1:
            # stage 0: bail out after phase A/B skeleton — just write h2T
            kvchk = ring.tile([128, 256], f32, tag="kvchk")
            nc.sync.dma_start(kvchk[:], kv_src[0:128, :])
            for m in range(nchunk):
                cn = min(CHUNK, npc - m * CHUNK)
                nc.sync.dma_start(outT_d[:, m * 128 : m * 128 + cn], h2T[:, m * 128 : m * 128 + cn])
            nc.sync.dma_start(outT_d[:, 0:128], kvchk[:, 0:128])
        _skip = stage < 1

        # ---------------- BN2 stats all-reduce
        stats = ring.tile([128, 2], f32, tag="stats")
        nc.vector.tensor_reduce(out=stats[:, 0:1], in_=s1p[:], op=OP.add, axis=mybir.AxisListType.X)
        nc.vector.tensor_reduce(out=stats[:, 1:2], in_=s2p[:], op=OP.add, axis=mybir.AxisListType.X)
        if c_cores > 1:
            st_in = dram.tile([128, 2], f32)
            st_out = nc.dram_tensor("st_out_sh", (128, 2), f32, kind="Internal", addr_space="Shared").ap()
            nc.sync.dma_start(st_in[:], stats[:])
            nc.gpsimd.collective_compute(
                "AllReduce",
                mybir.AluOpType.add,
                replica_groups=[list(range(c_cores))],
                ins=[st_in[:].opt()],
                outs=[st_out[:].opt()],
            )
            stg = ring.tile([128, 2], f32, tag="stg")
            nc.sync.dma_start(stg[:], st_out[:])
        else:
            stg = stats
        mean = ring.tile([128, 1], f32, tag="mean")
        nc.vector.tensor_scalar_mul(mean[:], stg[:, 0:1], 1.0 / n)
        ex2 = ring.tile([128, 1], f32, tag="ex2")
        nc.vector.tensor_scalar_mul(ex2[:], stg[:, 1:2], 1.0 / n)
        var = ring.tile([128, 1], f32, tag="var")
        nc.vector.tensor_tensor(out=var[:], in0=mean[:], in1=mean[:], op=OP.mult)
        nc.vector.tensor_tensor(out=var[:], in0=ex2[:], in1=var[:], op=OP.subtract)
        std = ring.tile([128, 1], f32, tag="std")
        nc.scalar.activation(out=std[:], in_=var[:], func=AF.Sqrt, bias=cvec[:, 5:6])
        rstd = ring.tile([128, 1], f32, tag="rstd")
        nc.vector.reciprocal(rstd[:], std[:])
        sc2 = ring.tile([128, 1], f32, tag="sc2")
        nc.vector.tensor_tensor(out=sc2[:], in0=rstd[:], in1=cvec[:, 4:5], op=OP.mult)

        # ---------------- phase C: BN2 apply + FFN + residual
        for m in range(nchunk):
            cn = min(CHUNK, npc - m * CHUNK)
            u = ring.tile([128, 128], f32, tag="u")
            nc.vector.scalar_tensor_tensor(
                out=u[:],
                in0=h2T[:, m * 128 : (m + 1) * 128],
                scalar=mean[:],
                op0=OP.subtract,
                in1=sc2[:].to_broadcast([128, 128]),
                op1=OP.mult,
            )
            y1a = psum.tile([128, 128], f32, tag="pa")
            nc.tensor.matmul(out=y1a[:], lhsT=w1[:, :128], rhs=u[:], start=True, stop=True)
            y1b = psum.tile([128, 128], f32, tag="qd")
            nc.tensor.matmul(out=y1b[:], lhsT=w1[:, 128:256], rhs=u[:], start=True, stop=True)
            r1a = ring.tile([128, 128], f32, tag="r1a")
            nc.scalar.activation(out=r1a[:], in_=y1a[:], func=AF.Relu, bias=cvec[:, 1:2])
            r1b = ring.tile([128, 128], f32, tag="r1b")
            nc.scalar.activation(out=r1b[:], in_=y1b[:], func=AF.Relu, bias=cvec[:, 2:3])
            h3 = psum.tile([128, 128], f32, tag="qd")
            nc.tensor.matmul(out=h3[:], lhsT=w2a[:], rhs=r1a[:], start=True, stop=False)
            nc.tensor.matmul(out=h3[:], lhsT=w2b[:], rhs=r1b[:], start=False, stop=True)
            outc = ring.tile([128, 128], f32, tag="outc")
            nc.vector.scalar_tensor_tensor(
                out=outc[:],
                in0=h3[:],
                scalar=cvec[:, 3:4],
                op0=OP.add,
                in1=h2T[:, m * 128 : (m + 1) * 128],
                op1=OP.add,
            )
            nc.sync.dma_start(outT_d[:, m * 128 : m * 128 + cn], outc[:, :cn])

    nc.compile()
    return nc


# ---------------------------------------------------------------- entry
def _make_cfg(n, e, c_cores, src, dst, b=GATHER_B):
    npc = n // c_cores
    nchunk = _ceil_div(npc, CHUNK)
    srcmeta, slotmeta, tpc, T, nloc = _prep_edges(src, dst, n, c_cores, npc, nchunk)
    cfg = dict(N=n, E=e, C=c_cores, NPC=npc, NCHUNK=nchunk, tpc=tpc, T=T, B=b, nloc=nloc)
    return cfg, srcmeta, slotmeta


def _make_in_maps(cfg, srcmeta, slotmeta, inp):
    f = np.float32
    n, c_cores, npc = cfg["N"], cfg["C"], cfg["NPC"]
    npad = cfg["NCHUNK"] * CHUNK
    w = _fold_weights(inp)
    h = np.asarray(inp["h"], f)
    in_maps = []
    for c in range(c_cores):
        hT = np.zeros((128, npad), f)
        hT[:, :npc] = h[c * npc : (c + 1) * npc, :].T
        m = dict(
            hT=hT, srcmeta=srcmeta[c], slotmeta=slotmeta[c],
            slotmetaT=np.ascontiguousarray(slotmeta[c].T).astype(np.float16).reshape(1, -1),
            **w,
        )
        in_maps.append(m)
    return in_maps


_CACHE = {}
_PROFILE = False
_LAST_RES = None


def kernel(**inputs):
    global _LAST_RES
    from concourse.bass_utils import run_bass_kernel_spmd

    src = np.asarray(inputs["src"]).astype(np.int32)
    dst = np.asarray(inputs["dst"]).astype(np.int32)
    cfg, srcmeta, slotmeta = _make_cfg(N, E, C, src, dst)
    key = ("full", tuple(cfg["tpc"]), tuple(cfg["nloc"]))
    if key not in _CACHE:
        _CACHE[key] = _build(cfg)
    nc = _CACHE[key]
    in_maps = _make_in_maps(cfg, srcmeta, slotmeta, inputs)
    res = run_bass_kernel_spmd(nc, in_maps, core_ids=list(range(C)), trace=_PROFILE)
    _LAST_RES = res
    npc = cfg["NPC"]
    out = np.empty((N, DIM), np.float32)
    for c in range(C):
        out[c * npc : (c + 1) * npc, :] = res.results[c]["outT"][:, :npc].T
    return out
